# revision 31
# baseline (speedup 1.0000x reference)
"""Multi-head causal attention (RoPE) for Trainium2, sharded over 8 NeuronCores.

v4 over v3:
 - interleaved qk partition layout [hA d0:32 | hB d0:32 | hA d32:64 | hB d32:64]
   so the RoPE half-rotation swap is 2 banded ops instead of 4; scores use
   4-way 32-row tile_position packing.
 - RoPE bias fused into scalar_tensor_tensor ops (no separate bias add).
 - V bias (bv) folded into the host-side gather (softmax weights sum to 1).
 - full-width contiguous exp (strided narrow exp was slower on ACT);
   affine-select and score/PV matmuls stay causally narrowed.
 - scores lookahead PRE=4 ahead of PV so the PE has independent work while
   the previous q-tile's normalization chain finishes (psO reuse).
"""
import numpy as np

B, S, D = 2, 2048, 1024
HEADS, HD = 16, 64
HALF = HD // 2
NCORES = 8
GROUPS = 4
HPG = HEADS // GROUPS
O_QK = 2 * HPG * HD   # 512
O_V = HPG * HD        # 256
SCALE = 1.0 / np.sqrt(HD)

NST = S // 512
NDC = D // 128
NKT = S // 128

_NC_CACHE = None
_BVADD = None

PSS_BUFS = 2
PSO_BUFS = 1
E_BUFS = 5
P1_BUFS = 3
PRE = 4
ROPE_ACT = True


def _build_nc():
    import concourse.bacc as bacc
    import concourse.mybir as mybir
    import concourse.tile as tile

    fp32 = mybir.dt.float32
    fp16 = mybir.dt.float16
    Exp = mybir.ActivationFunctionType.Exp
    ADD = mybir.AluOpType.add
    MULT = mybir.AluOpType.mult

    nc = bacc.Bacc(trn_type="TRN2", target_bir_lowering=False, debug=False)

    # all operands arrive pre-laid-out in their SBUF layouts so every input
    # DMA is a straight [128, N] contiguous copy (8-16KB runs per partition).
    xT = nc.dram_tensor("xT", [128, NDC * S], fp16, kind="ExternalInput").ap()
    wqkT = nc.dram_tensor("wqkT", [128, NDC * O_QK], fp16, kind="ExternalInput").ap()
    wvT = nc.dram_tensor("wvT", [128, NDC * O_V], fp16, kind="ExternalInput").ap()
    pwT = nc.dram_tensor("pwT", [128, 2 * D], fp16, kind="ExternalInput").ap()
    csF = nc.dram_tensor("csF", [128, 2 * S], fp16, kind="ExternalInput").ap()
    smalls = nc.dram_tensor("smalls", [128, 8], fp32, kind="ExternalInput").ap()
    outP = nc.dram_tensor("out_partial", [S, D], fp16, kind="ExternalOutput").ap()

    with tile.TileContext(nc) as tc:
        with tc.tile_pool(name="persist", bufs=1) as persist, \
             tc.tile_pool(name="p1sb", bufs=P1_BUFS) as p1sb, \
             tc.tile_pool(name="p2sb", bufs=E_BUFS) as p2sb, \
             tc.tile_pool(name="nrm", bufs=2) as nrm, \
             tc.tile_pool(name="p3sb", bufs=2) as p3sb, \
             tc.tile_pool(name="ps1", bufs=2, space="PSUM") as ps1_pool, \
             tc.tile_pool(name="psS", bufs=PSS_BUFS, space="PSUM") as psS_pool, \
             tc.tile_pool(name="psOA", bufs=PSO_BUFS, space="PSUM") as psOA_pool, \
             tc.tile_pool(name="psOB", bufs=PSO_BUFS, space="PSUM") as psOB_pool:

            # --- persistent tiles -------------------------------------------
            # x_all is chunk-major: [seq 0:512 | 512:1024 | 1024:2048], each
            # chunk dc-major, so chunked input DMAs stay fully contiguous.
            x_all = persist.tile([128, NDC * S], fp16, name="xall", tag="xall")
            wqk_all = persist.tile([128, NDC * O_QK], fp16, name="wqkall", tag="wqkall")
            wv_all = persist.tile([128, NDC * O_V], fp16, name="wvall", tag="wvall")
            pw_all = persist.tile([128, 2 * D], fp16, name="pwall", tag="pwall")

            def x_slice(dc, st):
                # 512 seq cols of contraction chunk dc for seq block st
                if st == 0:
                    off = dc * 512
                elif st == 1:
                    off = 4096 + dc * 512
                else:
                    off = 8192 + dc * 1024 + (st - 2) * 512
                return x_all[:, off:off + 512]

            wqk_t = [wqk_all[:, dc * O_QK:(dc + 1) * O_QK] for dc in range(NDC)]
            wv_t = [wv_all[:, dc * O_V:(dc + 1) * O_V] for dc in range(NDC)]
            pw_t = [pw_all[:, dc * D:(dc + 1) * D] for dc in range(2)]
            warm_w = persist.tile([128, 128], fp16, name="warmw", tag="warmw")
            warm_x = persist.tile([128, 512], fp16, name="warmx", tag="warmx")
            cs_t = persist.tile([128, 2 * S], fp16, name="cs", tag="cs")
            sm_t = persist.tile([128, 8], fp32, name="sm", tag="sm")
            qk_t = [persist.tile([128, S], fp16, name=f"qk{ot}", tag=f"qk{ot}")
                    for ot in range(4)]
            vaug_t = [persist.tile([128, HPG * (HD + 1)], fp16, name=f"va{kt}",
                                   tag=f"va{kt}")
                      for kt in range(NKT)]
            outT_t = [persist.tile([128, S], fp16, name=f"oT{t}", tag=f"oT{t}")
                      for t in range(2)]
            msk_t = persist.tile([128, 256], fp16, name="msk", tag="msk")

            # --- loads -------------------------------------------------------
            # HAM warm-up: dummy matmuls on a zeroed tile keep the PE busy
            # (and the clock at 8/8) while the input DMAs stream in.
            nc.vector.memset(warm_w[:], 0.0)
            nc.vector.memset(warm_x[:], 0.0)
            ps_warm = ps1_pool.tile([128, 512], fp32, name="pswarm", tag="ps1")
            N_WARM = 28
            for i in range(N_WARM):
                nc.tensor.matmul(ps_warm[:], warm_w[:], warm_x[:],
                                 start=(i == 0), stop=(i == N_WARM - 1))
            # consolidated loads: straight contiguous copies, one DMA each
            # (the SP queue serializes descriptor generation, so count matters).
            nc.sync.dma_start(out=wqk_all[:], in_=wqkT)
            nc.sync.dma_start(out=x_all[:, 0:4096], in_=xT[:, 0:4096])
            nc.sync.dma_start(out=cs_t[:], in_=csF)
            nc.sync.dma_start(out=sm_t[:], in_=smalls)
            nc.sync.dma_start(out=wv_all[:], in_=wvT)
            nc.sync.dma_start(out=x_all[:, 4096:8192], in_=xT[:, 4096:8192])
            nc.sync.dma_start(out=x_all[:, 8192:16384], in_=xT[:, 8192:16384])
            nc.sync.dma_start(out=pw_all[:], in_=pwT)
            for kt in range(NKT):
                nc.vector.memset(
                    vaug_t[kt].rearrange("p (h w) -> p h w", w=HD + 1)[:, :, HD:HD + 1],
                    1.0)
            # causal triangle mask for diagonal 128x128 blocks: 1 where q >= p,
            # duplicated side by side so one strided mul masks both heads.
            nc.vector.memset(msk_t[:], 1.0)
            for mh in range(2):
                nc.gpsimd.affine_select(
                    out=msk_t[:, mh * 128:(mh + 1) * 128],
                    in_=msk_t[:, mh * 128:(mh + 1) * 128],
                    compare_op=mybir.AluOpType.is_ge, fill=0.0,
                    base=0, pattern=[[1, 128]], channel_multiplier=-1)

            # --- phase-1 -----------------------------------------------------
            def emit_qk(st, ot):
                """One (o 128, s 512) q^T/k^T tile: matmul + fused bias + RoPE.

                Partition layout (L'): [hA d0:32 | hB d0:32 | hA d32:64 |
                hB d32:64]; rotation partner = p ^ 64.  out = (ps+b)*cos +
                swapped((ps+b)*sin'); sin' carries the sign at the read index.
                """
                s0 = st * 512
                ps = ps1_pool.tile([128, 512], fp32, name=f"psA{st}_{ot}", tag="ps1")
                for dc in range(NDC):
                    nc.tensor.matmul(ps[:], wqk_t[dc][:, ot * 128:(ot + 1) * 128],
                                     x_slice(dc, st),
                                     start=(dc == 0), stop=(dc == NDC - 1))
                if ROPE_ACT:
                    # bias-add on ACT (slack engine in phase1 windows); DVE
                    # does 3 fp16 tensor_tensor passes + the band-swap copies.
                    t = p1sb.tile([128, 512], fp16, name=f"t{st}_{ot}", tag="t")
                    nc.scalar.add(t[:], ps[:], sm_t[:, ot:ot + 1])
                    m2s = p1sb.tile([128, 512], fp16, name=f"m2s{st}_{ot}",
                                    tag="m2s")
                    nc.vector.tensor_mul(m2s[:], t[:], cs_t[:, S + s0:S + s0 + 512])
                else:
                    m2s = p1sb.tile([128, 512], fp16, name=f"m2s{st}_{ot}",
                                    tag="m2s")
                    nc.vector.scalar_tensor_tensor(
                        m2s[:], ps[:], sm_t[:, ot:ot + 1],
                        cs_t[:, S + s0:S + s0 + 512], op0=ADD, op1=MULT)
                m2 = p1sb.tile([128, 512], fp16, name=f"m2{st}_{ot}", tag="m2")
                for band in range(4):
                    dstp = band * 32
                    srcp = (band ^ 1) * 32
                    nc.vector.tensor_copy(m2[dstp:dstp + 32, :], m2s[srcp:srcp + 32, :])
                qs = qk_t[ot][:, s0:s0 + 512]
                if ROPE_ACT:
                    nc.vector.tensor_mul(qs, t[:], cs_t[:, s0:s0 + 512])
                else:
                    nc.vector.scalar_tensor_tensor(
                        qs, ps[:], sm_t[:, ot:ot + 1], cs_t[:, s0:s0 + 512],
                        op0=ADD, op1=MULT)
                nc.vector.tensor_add(qs, qs, m2[:])

            def emit_v(st, ss):
                ps = ps1_pool.tile([128, O_V], fp32, name=f"psB{st}_{ss}", tag="ps1")
                for dc in range(NDC):
                    nc.tensor.matmul(ps[:],
                                     x_slice(dc, st)[:, ss * 128:(ss + 1) * 128],
                                     wv_t[dc][:], start=(dc == 0), stop=(dc == NDC - 1))
                vt = vaug_t[st * 4 + ss]
                nc.vector.tensor_copy(
                    vt.rearrange("p (h w) -> p h w", w=HD + 1)[:, :, 0:HD],
                    ps.rearrange("p (h w) -> p h w", w=HD))

            def phase1_chunks(st):
                ch = []
                for ot in range(4):
                    ch.append(lambda ot=ot: emit_qk(st, ot))
                for ss in range(4):
                    ch.append(lambda ss=ss: emit_v(st, ss))
                return ch

            # --- attention ---------------------------------------------------
            def attention(qt, filler, tail_proj=None):
                q0 = qt * 512
                kmax = (q0 + 512) // 128
                n_fill_slots = 2 * (kmax + 2)
                fill_every = max(1, n_fill_slots // max(1, len(filler))) if filler else 0
                step = 0

                def maybe_fill():
                    nonlocal step
                    step += 1
                    if filler and fill_every and step % fill_every == 0:
                        filler.pop(0)()

                for hp in range(2):
                    q_tile = qk_t[hp]
                    k_tile = qk_t[2 + hp]
                    hA, hB = 2 * hp, 2 * hp + 1
                    psO_A = psOA_pool.tile([HD + 1, 512], fp32, name=f"psOA{hp}_{qt}",
                                           tag="psOA")
                    psO_B = psOB_pool.tile([HD + 1, 512], fp32, name=f"psOB{hp}_{qt}",
                                           tag="psOB")
                    e_tiles = [None] * kmax

                    def emit_st(kt):
                        k0 = kt * 128
                        lo = max(0, k0 - q0)
                        psS = psS_pool.tile([128, 1024], fp32, name=f"psS{hp}_{qt}_{kt}",
                                            tag="psS")
                        nc.tensor.matmul(psS[:, lo:512], k_tile[0:64, k0:k0 + 128],
                                         q_tile[0:64, q0 + lo:q0 + 512],
                                         start=True, stop=True, tile_position=(0, 0))
                        nc.tensor.matmul(psS[:, 512 + lo:1024], k_tile[64:128, k0:k0 + 128],
                                         q_tile[64:128, q0 + lo:q0 + 512],
                                         start=True, stop=True, tile_position=(64, 0))
                        e = p2sb.tile([128, 1024], fp16, name=f"e{hp}_{qt}_{kt}", tag="e")
                        nc.scalar.activation(e[:, lo:1024], psS[:, lo:1024],
                                             Exp, scale=float(SCALE))
                        if k0 > q0 - 128:
                            ev = e.rearrange("p (h w) -> p h w", h=2)[:, :, lo:lo + 128]
                            mv = msk_t.rearrange("p (h w) -> p h w", h=2)
                            nc.vector.tensor_mul(ev, ev, mv)
                        e_tiles[kt] = (e, lo)

                    def emit_pv(kt):
                        e, lo = e_tiles[kt]
                        nc.tensor.matmul(psO_A[:, lo:512],
                                         vaug_t[kt][:, hA * 65:hA * 65 + 65],
                                         e[:, lo:512], start=(kt == 0),
                                         stop=(kt == kmax - 1), skip_group_check=True)
                        nc.tensor.matmul(psO_B[:, lo:512],
                                         vaug_t[kt][:, hB * 65:hB * 65 + 65],
                                         e[:, 512 + lo:1024], start=(kt == 0),
                                         stop=(kt == kmax - 1), skip_group_check=True)

                    def norm_chunk(j0, w):
                        c0, c1 = j0 * 128, (j0 + w) * 128
                        for idx, psO in ((0, psO_A), (1, psO_B)):
                            lr = nrm.tile([1, 512], fp32,
                                          name=f"lr{hp}_{qt}_{idx}_{j0}", tag="lr")
                            nc.vector.tensor_copy(lr[0:1, 0:c1 - c0],
                                                  psO[HD:HD + 1, c0:c1])
                            rec = nrm.tile([1, 512], fp32,
                                           name=f"rec{hp}_{qt}_{idx}_{j0}", tag="rec")
                            nc.vector.reciprocal_approx_fast(
                                rec[0:1, 0:c1 - c0], lr[0:1, 0:c1 - c0])
                            rbc = nrm.tile([64, 512], fp32,
                                           name=f"rbc{hp}_{qt}_{idx}_{j0}", tag="rbc")
                            nc.gpsimd.partition_broadcast(
                                rbc[:, 0:c1 - c0], rec[0:1, 0:c1 - c0])
                            nc.vector.tensor_mul(
                                outT_t[hp][64 * idx:64 * idx + 64, q0 + c0:q0 + c1],
                                psO[0:64, c0:c1], rbc[:, 0:c1 - c0])

                    # scores lookahead: PE gets PRE tiles + fills of
                    # psO-independent work before the first PV touches psO.
                    emit_st(0)
                    emit_st(1)
                    maybe_fill()
                    if kmax > 2:
                        emit_st(2)
                    maybe_fill()
                    if kmax > 3:
                        emit_st(3)
                    for kt in range(kmax):
                        if kt + PRE < kmax:
                            emit_st(kt + PRE)
                        emit_pv(kt)
                        maybe_fill()
                    if tail_proj is not None and hp == 1:
                        # last q-block: normalize in halves with this block's
                        # proj interleaved, so the PE projects half 1 while the
                        # DVE/gpsimd normalize half 2 (nothing else follows).
                        norm_chunk(0, 2)
                        tail_proj[0]()
                        tail_proj[1]()
                        norm_chunk(2, 2)
                        tail_proj[2]()
                        tail_proj[3]()
                    else:
                        norm_chunk(0, 4)
                while filler:
                    filler.pop(0)()

            # --- output projection -------------------------------------------
            def emit_proj(st128):
                po = p3sb.tile([128, D], fp16, name=f"po{st128}", tag="po")
                for jt in range(2):
                    ps = ps1_pool.tile([128, 512], fp32, name=f"psP{st128}_{jt}",
                                       tag="ps1")
                    for dc in range(2):
                        nc.tensor.matmul(ps[:], outT_t[dc][:, st128 * 128:(st128 + 1) * 128],
                                         pw_t[dc][:, jt * 512:(jt + 1) * 512],
                                         start=(dc == 0), stop=(dc == 1))
                    # both evacuations on DVE: ACT is the exp-bound engine in the
                    # late windows where proj chunks now land.
                    nc.vector.tensor_copy(po[:, jt * 512:(jt + 1) * 512], ps[:])
                nc.sync.dma_start(
                    out=outP[st128 * 128:(st128 + 1) * 128, :], in_=po[:])

            def proj_chunks(qt):
                return [lambda sst=sst: emit_proj(qt * 4 + sst) for sst in range(4)]

            # --- main schedule ----------------------------------------------
            # Filler placement matched to per-window exp time (grows with qt):
            # early windows get only the mandatory phase1(qt+1); proj chunks
            # are deferred to the late, exp-heavy windows so the PE has work
            # while ACT drains the large score blocks.  attention(0) hp=0 only
            # needs qk ot 0/2 + v, so ot 1/3 ride as leading fillers.
            ch0 = phase1_chunks(0)
            pre0 = [ch0[0], ch0[2]] + ch0[4:8]
            fill_plan = {
                0: [ch0[1], ch0[3]] + phase1_chunks(1),
                1: phase1_chunks(2),
                2: phase1_chunks(3) + proj_chunks(0),
                3: proj_chunks(1) + proj_chunks(2),
            }
            for ch in pre0:
                ch()
            for qt in range(NST):
                attention(qt, fill_plan[qt],
                          tail_proj=proj_chunks(qt) if qt == NST - 1 else None)

    nc.compile()
    return nc


def _get_nc():
    global _NC_CACHE
    if _NC_CACHE is None:
        _NC_CACHE = _build_nc()
    return _NC_CACHE


def _prep_in_maps(x, qkv_w, qkv_b, proj_w, proj_b):
    x = np.asarray(x, dtype=np.float32)
    qkv_w = np.asarray(qkv_w, dtype=np.float32)
    qkv_b = np.asarray(qkv_b, dtype=np.float32)
    proj_w = np.asarray(proj_w, dtype=np.float32)

    # RoPE tables in the interleaved (L') layout: freq index = p % 32,
    # sign = +1 for p < 64 (d < 32), -1 for p >= 64 (d >= 32).
    freqs = (1.0 / (10000.0 ** (np.arange(HALF, dtype=np.float32) / HALF))).astype(np.float32)
    pos = np.arange(S, dtype=np.float32)
    ang = pos[:, None] * freqs[None, :]            # (S, 32)
    cos_m = np.cos(ang).astype(np.float32)
    sin_m = np.sin(ang).astype(np.float32)
    p = np.arange(128)
    cosF = np.ascontiguousarray(cos_m[:, p % HALF].T)            # (128, S)
    sign = np.where((p % HD) < HALF, 1.0, -1.0).astype(np.float32)
    sinF = np.ascontiguousarray((sin_m[:, p % HALF] * sign[None, :]).T)
    csF = np.concatenate([cosF, sinF], axis=1).astype(np.float16)

    global _BVADD
    _BVADD = proj_w @ qkv_b[2 * D:3 * D]

    def chunk_major(a, widths):
        # [D_rows, C] -> [128, sum(n_chunks*width)]: per seq/col chunk,
        # contraction-chunk-major, partition-contiguous (fast DMA layout)
        blocks = []
        off = 0
        for w in widths:
            nchunks = a.shape[0] // 128
            blk = a[:, off:off + w].reshape(nchunks, 128, w)
            blocks.append(blk.transpose(1, 0, 2).reshape(128, nchunks * w))
            off += w
        return np.ascontiguousarray(np.concatenate(blocks, axis=1)).astype(np.float16)

    xb_l = [chunk_major(x[b].T, [512, 512, 1024]) for b in range(B)]
    in_maps = []
    for c in range(NCORES):
        b, g = divmod(c, GROUPS)
        hs = np.arange(HPG) + HPG * g
        q_rows = np.concatenate([np.arange(HD * h, HD * h + HD) for h in hs])
        v_rows = q_rows + 2 * D
        qk_rows = np.concatenate([q_rows, q_rows + D])
        smalls = np.zeros((128, 8), dtype=np.float32)
        smalls[:, 0:4] = qkv_b[qk_rows].reshape(4, 128).T
        in_maps.append({
            "xT": xb_l[b],
            "wqkT": chunk_major(qkv_w[qk_rows, :].T, [O_QK]),
            "wvT": chunk_major(qkv_w[v_rows, :].T, [O_V]),
            "pwT": chunk_major(proj_w[:, q_rows].T, [D]),
            "csF": csF,
            "smalls": smalls,
        })
    return in_maps


def _gather(results, proj_b):
    # v-bias folded here: softmax weights sum to 1, so attention(v + bv) =
    # attention(v) + bv, and proj(attn + bv) = proj(attn) + proj_w @ bv.
    proj_b = np.asarray(proj_b, dtype=np.float32)
    out = np.empty((B, S, D), dtype=np.float32)
    add = proj_b + _BVADD
    for b in range(B):
        acc = np.zeros((S, D), dtype=np.float32)
        for g in range(GROUPS):
            acc += results[b * GROUPS + g]["out_partial"].astype(np.float32)
        out[b] = acc + add[None, :]
    return out


def _run(x, qkv_w, qkv_b, proj_w, proj_b, trace=False):
    from concourse import bass_utils
    nc = _get_nc()
    in_maps = _prep_in_maps(x, qkv_w, qkv_b, proj_w, proj_b)
    res = bass_utils.run_bass_kernel_spmd(nc, in_maps, core_ids=list(range(NCORES)),
                                          trace=trace)
    return _gather(res.results, proj_b), res


def kernel(x, qkv_w, qkv_b, proj_w, proj_b):
    out, _ = _run(x, qkv_w, qkv_b, proj_w, proj_b, trace=False)
    return out



# revision 33
# speedup vs baseline: 1.0115x; 1.0115x over previous
"""Multi-head causal attention (RoPE) for Trainium2, sharded over 8 NeuronCores.

v4 over v3:
 - interleaved qk partition layout [hA d0:32 | hB d0:32 | hA d32:64 | hB d32:64]
   so the RoPE half-rotation swap is 2 banded ops instead of 4; scores use
   4-way 32-row tile_position packing.
 - RoPE bias fused into scalar_tensor_tensor ops (no separate bias add).
 - V bias (bv) folded into the host-side gather (softmax weights sum to 1).
 - full-width contiguous exp (strided narrow exp was slower on ACT);
   affine-select and score/PV matmuls stay causally narrowed.
 - scores lookahead PRE=4 ahead of PV so the PE has independent work while
   the previous q-tile's normalization chain finishes (psO reuse).
"""
import numpy as np

B, S, D = 2, 2048, 1024
HEADS, HD = 16, 64
HALF = HD // 2
NCORES = 8
GROUPS = 4
HPG = HEADS // GROUPS
O_QK = 2 * HPG * HD   # 512
O_V = HPG * HD        # 256
SCALE = 1.0 / np.sqrt(HD)

NST = S // 512
NDC = D // 128
NKT = S // 128

_NC_CACHE = None
_BVADD = None

PSS_BUFS = 2
PSO_BUFS = 1
E_BUFS = 5
P1_BUFS = 3
PRE = 4
ROPE_ACT = True


def _build_nc():
    import concourse.bacc as bacc
    import concourse.mybir as mybir
    import concourse.tile as tile

    fp32 = mybir.dt.float32
    fp16 = mybir.dt.float16
    Exp = mybir.ActivationFunctionType.Exp
    ADD = mybir.AluOpType.add
    MULT = mybir.AluOpType.mult

    nc = bacc.Bacc(trn_type="TRN2", target_bir_lowering=False, debug=False)

    # all operands arrive pre-laid-out in their SBUF layouts so every input
    # DMA is a straight [128, N] contiguous copy (8-16KB runs per partition).
    xT = nc.dram_tensor("xT", [128, NDC * S], fp16, kind="ExternalInput").ap()
    wqkT = nc.dram_tensor("wqkT", [128, NDC * O_QK], fp16, kind="ExternalInput").ap()
    wvT = nc.dram_tensor("wvT", [128, NDC * O_V], fp16, kind="ExternalInput").ap()
    pwT = nc.dram_tensor("pwT", [128, 2 * D], fp16, kind="ExternalInput").ap()
    csF = nc.dram_tensor("csF", [128, 2 * S], fp16, kind="ExternalInput").ap()
    smalls = nc.dram_tensor("smalls", [128, 8], fp32, kind="ExternalInput").ap()
    outP = nc.dram_tensor("out_partial", [S, D], fp16, kind="ExternalOutput").ap()

    with tile.TileContext(nc) as tc:
        with tc.tile_pool(name="persist", bufs=1) as persist, \
             tc.tile_pool(name="p1sb", bufs=P1_BUFS) as p1sb, \
             tc.tile_pool(name="p2sb", bufs=E_BUFS) as p2sb, \
             tc.tile_pool(name="nrm", bufs=2) as nrm, \
             tc.tile_pool(name="p3sb", bufs=2) as p3sb, \
             tc.tile_pool(name="ps1", bufs=2, space="PSUM") as ps1_pool, \
             tc.tile_pool(name="psS", bufs=PSS_BUFS, space="PSUM") as psS_pool, \
             tc.tile_pool(name="psOA", bufs=PSO_BUFS, space="PSUM") as psOA_pool, \
             tc.tile_pool(name="psOB", bufs=PSO_BUFS, space="PSUM") as psOB_pool:

            # --- persistent tiles -------------------------------------------
            # x_all is chunk-major: [seq 0:512 | 512:1024 | 1024:2048], each
            # chunk dc-major, so chunked input DMAs stay fully contiguous.
            x_all = persist.tile([128, NDC * S], fp16, name="xall", tag="xall")
            wqk_all = persist.tile([128, NDC * O_QK], fp16, name="wqkall", tag="wqkall")
            wv_all = persist.tile([128, NDC * O_V], fp16, name="wvall", tag="wvall")
            pw_all = persist.tile([128, 2 * D], fp16, name="pwall", tag="pwall")

            def x_slice(dc, st):
                # 512 seq cols of contraction chunk dc for seq block st
                if st == 0:
                    off = dc * 512
                elif st == 1:
                    off = 4096 + dc * 512
                else:
                    off = 8192 + dc * 1024 + (st - 2) * 512
                return x_all[:, off:off + 512]

            wqk_t = [wqk_all[:, dc * O_QK:(dc + 1) * O_QK] for dc in range(NDC)]
            wv_t = [wv_all[:, dc * O_V:(dc + 1) * O_V] for dc in range(NDC)]
            pw_t = [pw_all[:, dc * D:(dc + 1) * D] for dc in range(2)]
            warm_w = persist.tile([128, 128], fp16, name="warmw", tag="warmw")
            warm_x = persist.tile([128, 512], fp16, name="warmx", tag="warmx")
            cs_t = persist.tile([128, 2 * S], fp16, name="cs", tag="cs")
            sm_t = persist.tile([128, 8], fp32, name="sm", tag="sm")
            qk_t = [persist.tile([128, S], fp16, name=f"qk{ot}", tag=f"qk{ot}")
                    for ot in range(4)]
            vaug_t = [persist.tile([128, HPG * (HD + 1)], fp16, name=f"va{kt}",
                                   tag=f"va{kt}")
                      for kt in range(NKT)]
            outT_t = [persist.tile([128, S], fp16, name=f"oT{t}", tag=f"oT{t}")
                      for t in range(2)]
            msk_t = persist.tile([128, 256], fp16, name="msk", tag="msk")

            # --- loads -------------------------------------------------------
            # HAM warm-up: dummy matmuls on a zeroed tile keep the PE busy
            # (and the clock at 8/8) while the input DMAs stream in.
            nc.vector.memset(warm_w[:], 0.0)
            nc.vector.memset(warm_x[:], 0.0)
            ps_warm = ps1_pool.tile([128, 512], fp32, name="pswarm", tag="ps1")
            N_WARM = 28
            for i in range(N_WARM):
                nc.tensor.matmul(ps_warm[:], warm_w[:], warm_x[:],
                                 start=(i == 0), stop=(i == N_WARM - 1))
            # consolidated loads: straight contiguous copies, one DMA each
            # (the SP queue serializes descriptor generation, so count matters).
            nc.sync.dma_start(out=wqk_all[:], in_=wqkT)
            nc.sync.dma_start(out=x_all[:, 0:4096], in_=xT[:, 0:4096])
            nc.sync.dma_start(out=cs_t[:], in_=csF)
            nc.sync.dma_start(out=sm_t[:], in_=smalls)
            nc.sync.dma_start(out=wv_all[:], in_=wvT)
            nc.sync.dma_start(out=x_all[:, 4096:8192], in_=xT[:, 4096:8192])
            nc.sync.dma_start(out=x_all[:, 8192:16384], in_=xT[:, 8192:16384])
            nc.sync.dma_start(out=pw_all[:], in_=pwT)
            for kt in range(NKT):
                nc.vector.memset(
                    vaug_t[kt].rearrange("p (h w) -> p h w", w=HD + 1)[:, :, HD:HD + 1],
                    1.0)
            # causal triangle mask for diagonal 128x128 blocks: 1 where q >= p,
            # duplicated side by side so one strided mul masks both heads.
            nc.vector.memset(msk_t[:], 1.0)
            for mh in range(2):
                nc.gpsimd.affine_select(
                    out=msk_t[:, mh * 128:(mh + 1) * 128],
                    in_=msk_t[:, mh * 128:(mh + 1) * 128],
                    compare_op=mybir.AluOpType.is_ge, fill=0.0,
                    base=0, pattern=[[1, 128]], channel_multiplier=-1)

            # --- phase-1 -----------------------------------------------------
            def emit_qk(st, ot):
                """One (o 128, s 512) q^T/k^T tile: matmul + fused bias + RoPE.

                Partition layout (L'): [hA d0:32 | hB d0:32 | hA d32:64 |
                hB d32:64]; rotation partner = p ^ 64.  out = (ps+b)*cos +
                swapped((ps+b)*sin'); sin' carries the sign at the read index.
                """
                s0 = st * 512
                ps = ps1_pool.tile([128, 512], fp32, name=f"psA{st}_{ot}", tag="ps1")
                for dc in range(NDC):
                    nc.tensor.matmul(ps[:], wqk_t[dc][:, ot * 128:(ot + 1) * 128],
                                     x_slice(dc, st),
                                     start=(dc == 0), stop=(dc == NDC - 1))
                if ROPE_ACT:
                    # bias-add on ACT (slack engine in phase1 windows); DVE
                    # does 3 fp16 tensor_tensor passes + the band-swap copies.
                    t = p1sb.tile([128, 512], fp16, name=f"t{st}_{ot}", tag="t")
                    nc.scalar.add(t[:], ps[:], sm_t[:, ot:ot + 1])
                    m2s = p1sb.tile([128, 512], fp16, name=f"m2s{st}_{ot}",
                                    tag="m2s")
                    nc.vector.tensor_mul(m2s[:], t[:], cs_t[:, S + s0:S + s0 + 512])
                else:
                    m2s = p1sb.tile([128, 512], fp16, name=f"m2s{st}_{ot}",
                                    tag="m2s")
                    nc.vector.scalar_tensor_tensor(
                        m2s[:], ps[:], sm_t[:, ot:ot + 1],
                        cs_t[:, S + s0:S + s0 + 512], op0=ADD, op1=MULT)
                m2 = p1sb.tile([128, 512], fp16, name=f"m2{st}_{ot}", tag="m2")
                for band in range(4):
                    dstp = band * 32
                    srcp = (band ^ 1) * 32
                    nc.vector.tensor_copy(m2[dstp:dstp + 32, :], m2s[srcp:srcp + 32, :])
                qs = qk_t[ot][:, s0:s0 + 512]
                if ROPE_ACT:
                    nc.vector.tensor_mul(qs, t[:], cs_t[:, s0:s0 + 512])
                else:
                    nc.vector.scalar_tensor_tensor(
                        qs, ps[:], sm_t[:, ot:ot + 1], cs_t[:, s0:s0 + 512],
                        op0=ADD, op1=MULT)
                nc.vector.tensor_add(qs, qs, m2[:])

            def emit_v(st, ss):
                ps = ps1_pool.tile([128, O_V], fp32, name=f"psB{st}_{ss}", tag="ps1")
                for dc in range(NDC):
                    nc.tensor.matmul(ps[:],
                                     x_slice(dc, st)[:, ss * 128:(ss + 1) * 128],
                                     wv_t[dc][:], start=(dc == 0), stop=(dc == NDC - 1))
                vt = vaug_t[st * 4 + ss]
                nc.vector.tensor_copy(
                    vt.rearrange("p (h w) -> p h w", w=HD + 1)[:, :, 0:HD],
                    ps.rearrange("p (h w) -> p h w", w=HD))

            def phase1_chunks(st):
                ch = []
                for ot in range(4):
                    ch.append(lambda ot=ot: emit_qk(st, ot))
                for ss in range(4):
                    ch.append(lambda ss=ss: emit_v(st, ss))
                return ch

            # --- attention ---------------------------------------------------
            def attention(qt, filler, tail_proj=None):
                q0 = qt * 512
                kmax = (q0 + 512) // 128
                n_fill_slots = 2 * (kmax + 2)
                fill_every = max(1, n_fill_slots // max(1, len(filler))) if filler else 0
                step = 0

                def maybe_fill():
                    nonlocal step
                    step += 1
                    if filler and fill_every and step % fill_every == 0:
                        filler.pop(0)()

                for hp in range(2):
                    q_tile = qk_t[hp]
                    k_tile = qk_t[2 + hp]
                    hA, hB = 2 * hp, 2 * hp + 1
                    psO_A = psOA_pool.tile([HD + 1, 512], fp32, name=f"psOA{hp}_{qt}",
                                           tag="psOA")
                    psO_B = psOB_pool.tile([HD + 1, 512], fp32, name=f"psOB{hp}_{qt}",
                                           tag="psOB")
                    e_tiles = [None] * kmax

                    def emit_st(kt):
                        k0 = kt * 128
                        lo = max(0, k0 - q0)
                        psS = psS_pool.tile([128, 1024], fp32, name=f"psS{hp}_{qt}_{kt}",
                                            tag="psS")
                        nc.tensor.matmul(psS[:, lo:512], k_tile[0:64, k0:k0 + 128],
                                         q_tile[0:64, q0 + lo:q0 + 512],
                                         start=True, stop=True, tile_position=(0, 0))
                        nc.tensor.matmul(psS[:, 512 + lo:1024], k_tile[64:128, k0:k0 + 128],
                                         q_tile[64:128, q0 + lo:q0 + 512],
                                         start=True, stop=True, tile_position=(64, 0))
                        e = p2sb.tile([128, 1024], fp16, name=f"e{hp}_{qt}_{kt}", tag="e")
                        nc.scalar.activation(e[:, lo:1024], psS[:, lo:1024],
                                             Exp, scale=float(SCALE))
                        if k0 > q0 - 128:
                            ev = e.rearrange("p (h w) -> p h w", h=2)[:, :, lo:lo + 128]
                            mv = msk_t.rearrange("p (h w) -> p h w", h=2)
                            nc.vector.tensor_mul(ev, ev, mv)
                        e_tiles[kt] = (e, lo)

                    def emit_pv(kt):
                        e, lo = e_tiles[kt]
                        nc.tensor.matmul(psO_A[:, lo:512],
                                         vaug_t[kt][:, hA * 65:hA * 65 + 65],
                                         e[:, lo:512], start=(kt == 0),
                                         stop=(kt == kmax - 1), skip_group_check=True)
                        nc.tensor.matmul(psO_B[:, lo:512],
                                         vaug_t[kt][:, hB * 65:hB * 65 + 65],
                                         e[:, 512 + lo:1024], start=(kt == 0),
                                         stop=(kt == kmax - 1), skip_group_check=True)

                    def norm_chunk(j0, w):
                        c0, c1 = j0 * 128, (j0 + w) * 128
                        for idx, psO in ((0, psO_A), (1, psO_B)):
                            lr = nrm.tile([1, 512], fp32,
                                          name=f"lr{hp}_{qt}_{idx}_{j0}", tag="lr")
                            nc.vector.tensor_copy(lr[0:1, 0:c1 - c0],
                                                  psO[HD:HD + 1, c0:c1])
                            rec = nrm.tile([1, 512], fp32,
                                           name=f"rec{hp}_{qt}_{idx}_{j0}", tag="rec")
                            nc.vector.reciprocal_approx_fast(
                                rec[0:1, 0:c1 - c0], lr[0:1, 0:c1 - c0])
                            rbc = nrm.tile([64, 512], fp32,
                                           name=f"rbc{hp}_{qt}_{idx}_{j0}", tag="rbc")
                            nc.gpsimd.partition_broadcast(
                                rbc[:, 0:c1 - c0], rec[0:1, 0:c1 - c0])
                            nc.vector.tensor_mul(
                                outT_t[hp][64 * idx:64 * idx + 64, q0 + c0:q0 + c1],
                                psO[0:64, c0:c1], rbc[:, 0:c1 - c0])

                    # scores lookahead: PE gets PRE tiles + fills of
                    # psO-independent work before the first PV touches psO.
                    emit_st(0)
                    emit_st(1)
                    maybe_fill()
                    if kmax > 2:
                        emit_st(2)
                    maybe_fill()
                    if kmax > 3:
                        emit_st(3)
                    for kt in range(kmax):
                        if kt + PRE < kmax:
                            emit_st(kt + PRE)
                        emit_pv(kt)
                        maybe_fill()
                    if tail_proj is not None and hp == 1:
                        # last q-block: normalize in halves with this block's
                        # proj interleaved, so the PE projects half 1 while the
                        # DVE/gpsimd normalize half 2 (nothing else follows).
                        norm_chunk(0, 2)
                        tail_proj[0]()
                        tail_proj[1]()
                        norm_chunk(2, 2)
                        tail_proj[2]()
                        tail_proj[3]()
                    else:
                        norm_chunk(0, 4)
                while filler:
                    filler.pop(0)()

            # --- output projection -------------------------------------------
            def emit_proj(st128, on_act=False):
                po = p3sb.tile([128, D], fp16, name=f"po{st128}", tag="po")
                for jt in range(2):
                    ps = ps1_pool.tile([128, 512], fp32, name=f"psP{st128}_{jt}",
                                       tag="ps1")
                    for dc in range(2):
                        nc.tensor.matmul(ps[:], outT_t[dc][:, st128 * 128:(st128 + 1) * 128],
                                         pw_t[dc][:, jt * 512:(jt + 1) * 512],
                                         start=(dc == 0), stop=(dc == 1))
                    # mid-kernel evacuations go to DVE (ACT is exp-bound there);
                    # the tail chunks use ACT, which is idle once exp is done.
                    if on_act:
                        nc.scalar.copy(po[:, jt * 512:(jt + 1) * 512], ps[:])
                    else:
                        nc.vector.tensor_copy(po[:, jt * 512:(jt + 1) * 512], ps[:])
                nc.sync.dma_start(
                    out=outP[st128 * 128:(st128 + 1) * 128, :], in_=po[:])

            def proj_chunks(qt, on_act=False):
                return [lambda sst=sst: emit_proj(qt * 4 + sst, on_act)
                        for sst in range(4)]

            # --- main schedule ----------------------------------------------
            # Filler placement matched to per-window exp time (grows with qt):
            # early windows get only the mandatory phase1(qt+1); proj chunks
            # are deferred to the late, exp-heavy windows so the PE has work
            # while ACT drains the large score blocks.  attention(0) hp=0 only
            # needs qk ot 0/2 + v, so ot 1/3 ride as leading fillers.
            ch0 = phase1_chunks(0)
            pre0 = [ch0[0], ch0[2]] + ch0[4:8]
            fill_plan = {
                0: [ch0[1], ch0[3]] + phase1_chunks(1),
                1: phase1_chunks(2),
                2: phase1_chunks(3) + proj_chunks(0),
                3: proj_chunks(1) + proj_chunks(2),
            }
            for ch in pre0:
                ch()
            for qt in range(NST):
                attention(qt, fill_plan[qt],
                          tail_proj=(proj_chunks(qt, on_act=True)
                                     if qt == NST - 1 else None))

    nc.compile()
    return nc


def _get_nc():
    global _NC_CACHE
    if _NC_CACHE is None:
        _NC_CACHE = _build_nc()
    return _NC_CACHE


def _prep_in_maps(x, qkv_w, qkv_b, proj_w, proj_b):
    x = np.asarray(x, dtype=np.float32)
    qkv_w = np.asarray(qkv_w, dtype=np.float32)
    qkv_b = np.asarray(qkv_b, dtype=np.float32)
    proj_w = np.asarray(proj_w, dtype=np.float32)

    # RoPE tables in the interleaved (L') layout: freq index = p % 32,
    # sign = +1 for p < 64 (d < 32), -1 for p >= 64 (d >= 32).
    freqs = (1.0 / (10000.0 ** (np.arange(HALF, dtype=np.float32) / HALF))).astype(np.float32)
    pos = np.arange(S, dtype=np.float32)
    ang = pos[:, None] * freqs[None, :]            # (S, 32)
    cos_m = np.cos(ang).astype(np.float32)
    sin_m = np.sin(ang).astype(np.float32)
    p = np.arange(128)
    cosF = np.ascontiguousarray(cos_m[:, p % HALF].T)            # (128, S)
    sign = np.where((p % HD) < HALF, 1.0, -1.0).astype(np.float32)
    sinF = np.ascontiguousarray((sin_m[:, p % HALF] * sign[None, :]).T)
    csF = np.concatenate([cosF, sinF], axis=1).astype(np.float16)

    global _BVADD
    _BVADD = proj_w @ qkv_b[2 * D:3 * D]

    def chunk_major(a, widths):
        # [D_rows, C] -> [128, sum(n_chunks*width)]: per seq/col chunk,
        # contraction-chunk-major, partition-contiguous (fast DMA layout)
        blocks = []
        off = 0
        for w in widths:
            nchunks = a.shape[0] // 128
            blk = a[:, off:off + w].reshape(nchunks, 128, w)
            blocks.append(blk.transpose(1, 0, 2).reshape(128, nchunks * w))
            off += w
        return np.ascontiguousarray(np.concatenate(blocks, axis=1)).astype(np.float16)

    xb_l = [chunk_major(x[b].T, [512, 512, 1024]) for b in range(B)]
    in_maps = []
    for c in range(NCORES):
        b, g = divmod(c, GROUPS)
        hs = np.arange(HPG) + HPG * g
        q_rows = np.concatenate([np.arange(HD * h, HD * h + HD) for h in hs])
        v_rows = q_rows + 2 * D
        qk_rows = np.concatenate([q_rows, q_rows + D])
        smalls = np.zeros((128, 8), dtype=np.float32)
        smalls[:, 0:4] = qkv_b[qk_rows].reshape(4, 128).T
        in_maps.append({
            "xT": xb_l[b],
            "wqkT": chunk_major(qkv_w[qk_rows, :].T, [O_QK]),
            "wvT": chunk_major(qkv_w[v_rows, :].T, [O_V]),
            "pwT": chunk_major(proj_w[:, q_rows].T, [D]),
            "csF": csF,
            "smalls": smalls,
        })
    return in_maps


def _gather(results, proj_b):
    # v-bias folded here: softmax weights sum to 1, so attention(v + bv) =
    # attention(v) + bv, and proj(attn + bv) = proj(attn) + proj_w @ bv.
    proj_b = np.asarray(proj_b, dtype=np.float32)
    out = np.empty((B, S, D), dtype=np.float32)
    add = proj_b + _BVADD
    for b in range(B):
        acc = np.zeros((S, D), dtype=np.float32)
        for g in range(GROUPS):
            acc += results[b * GROUPS + g]["out_partial"].astype(np.float32)
        out[b] = acc + add[None, :]
    return out


def _run(x, qkv_w, qkv_b, proj_w, proj_b, trace=False):
    from concourse import bass_utils
    nc = _get_nc()
    in_maps = _prep_in_maps(x, qkv_w, qkv_b, proj_w, proj_b)
    res = bass_utils.run_bass_kernel_spmd(nc, in_maps, core_ids=list(range(NCORES)),
                                          trace=trace)
    return _gather(res.results, proj_b), res


def kernel(x, qkv_w, qkv_b, proj_w, proj_b):
    out, _ = _run(x, qkv_w, qkv_b, proj_w, proj_b, trace=False)
    return out



# revision 34
# speedup vs baseline: 1.0127x; 1.0013x over previous
"""Multi-head causal attention (RoPE) for Trainium2, sharded over 8 NeuronCores.

v8 over v4 (176.7us -> ~168us):
 - exp-aware filler schedule: per-window exp time grows with qt, so proj
   chunks are deferred to the late, exp-heavy windows and phase1(qt+1) is the
   only early filler; attention(0) starts after just qk ot0/ot2 + v chunks.
 - RoPE bias-add moved to ACT (scalar.add from PSUM), leaving DVE with three
   fp16 tensor_tensor passes + the band-swap copies (~2.1us/tile vs 2.9).
 - diagonal causal masking as ONE strided DVE mul over both packed heads.
 - proj PSUM evacuation on DVE mid-kernel (ACT is exp-bound there) but on
   ACT for the tail chunks (ACT idle once exp drains).
 - last q-block: normalization in halves with its proj interleaved, so the
   PE projects half 1 while DVE/gpsimd normalize half 2.
 - all inputs host-prelaid into their exact SBUF layouts (x chunk-major) so
   every input DMA is a straight [128, N] contiguous copy.
 - V bias folded into the host-side gather; full-width contiguous exp;
   scores lookahead PRE=4; score/PV matmuls causally narrowed.
"""
import numpy as np

B, S, D = 2, 2048, 1024
HEADS, HD = 16, 64
HALF = HD // 2
NCORES = 8
GROUPS = 4
HPG = HEADS // GROUPS
O_QK = 2 * HPG * HD   # 512
O_V = HPG * HD        # 256
SCALE = 1.0 / np.sqrt(HD)

NST = S // 512
NDC = D // 128
NKT = S // 128

_NC_CACHE = None
_BVADD = None

PSS_BUFS = 2
PSO_BUFS = 1
E_BUFS = 5
P1_BUFS = 3
PRE = 4
ROPE_ACT = True


def _build_nc():
    import concourse.bacc as bacc
    import concourse.mybir as mybir
    import concourse.tile as tile

    fp32 = mybir.dt.float32
    fp16 = mybir.dt.float16
    Exp = mybir.ActivationFunctionType.Exp
    ADD = mybir.AluOpType.add
    MULT = mybir.AluOpType.mult

    nc = bacc.Bacc(trn_type="TRN2", target_bir_lowering=False, debug=False)

    # all operands arrive pre-laid-out in their SBUF layouts so every input
    # DMA is a straight [128, N] contiguous copy (8-16KB runs per partition).
    xT = nc.dram_tensor("xT", [128, NDC * S], fp16, kind="ExternalInput").ap()
    wqkT = nc.dram_tensor("wqkT", [128, NDC * O_QK], fp16, kind="ExternalInput").ap()
    wvT = nc.dram_tensor("wvT", [128, NDC * O_V], fp16, kind="ExternalInput").ap()
    pwT = nc.dram_tensor("pwT", [128, 2 * D], fp16, kind="ExternalInput").ap()
    csF = nc.dram_tensor("csF", [128, 2 * S], fp16, kind="ExternalInput").ap()
    smalls = nc.dram_tensor("smalls", [128, 8], fp32, kind="ExternalInput").ap()
    outP = nc.dram_tensor("out_partial", [S, D], fp16, kind="ExternalOutput").ap()

    with tile.TileContext(nc) as tc:
        with tc.tile_pool(name="persist", bufs=1) as persist, \
             tc.tile_pool(name="p1sb", bufs=P1_BUFS) as p1sb, \
             tc.tile_pool(name="p2sb", bufs=E_BUFS) as p2sb, \
             tc.tile_pool(name="nrm", bufs=2) as nrm, \
             tc.tile_pool(name="p3sb", bufs=2) as p3sb, \
             tc.tile_pool(name="ps1", bufs=2, space="PSUM") as ps1_pool, \
             tc.tile_pool(name="psS", bufs=PSS_BUFS, space="PSUM") as psS_pool, \
             tc.tile_pool(name="psOA", bufs=PSO_BUFS, space="PSUM") as psOA_pool, \
             tc.tile_pool(name="psOB", bufs=PSO_BUFS, space="PSUM") as psOB_pool:

            # --- persistent tiles -------------------------------------------
            # x_all is chunk-major: [seq 0:512 | 512:1024 | 1024:2048], each
            # chunk dc-major, so chunked input DMAs stay fully contiguous.
            x_all = persist.tile([128, NDC * S], fp16, name="xall", tag="xall")
            wqk_all = persist.tile([128, NDC * O_QK], fp16, name="wqkall", tag="wqkall")
            wv_all = persist.tile([128, NDC * O_V], fp16, name="wvall", tag="wvall")
            pw_all = persist.tile([128, 2 * D], fp16, name="pwall", tag="pwall")

            def x_slice(dc, st):
                # 512 seq cols of contraction chunk dc for seq block st
                if st == 0:
                    off = dc * 512
                elif st == 1:
                    off = 4096 + dc * 512
                else:
                    off = 8192 + dc * 1024 + (st - 2) * 512
                return x_all[:, off:off + 512]

            wqk_t = [wqk_all[:, dc * O_QK:(dc + 1) * O_QK] for dc in range(NDC)]
            wv_t = [wv_all[:, dc * O_V:(dc + 1) * O_V] for dc in range(NDC)]
            pw_t = [pw_all[:, dc * D:(dc + 1) * D] for dc in range(2)]
            warm_w = persist.tile([128, 128], fp16, name="warmw", tag="warmw")
            warm_x = persist.tile([128, 512], fp16, name="warmx", tag="warmx")
            cs_t = persist.tile([128, 2 * S], fp16, name="cs", tag="cs")
            sm_t = persist.tile([128, 8], fp32, name="sm", tag="sm")
            qk_t = [persist.tile([128, S], fp16, name=f"qk{ot}", tag=f"qk{ot}")
                    for ot in range(4)]
            vaug_t = [persist.tile([128, HPG * (HD + 1)], fp16, name=f"va{kt}",
                                   tag=f"va{kt}")
                      for kt in range(NKT)]
            outT_t = [persist.tile([128, S], fp16, name=f"oT{t}", tag=f"oT{t}")
                      for t in range(2)]
            msk_t = persist.tile([128, 256], fp16, name="msk", tag="msk")

            # --- loads -------------------------------------------------------
            # HAM warm-up: dummy matmuls on a zeroed tile keep the PE busy
            # (and the clock at 8/8) while the input DMAs stream in.
            nc.vector.memset(warm_w[:], 0.0)
            nc.vector.memset(warm_x[:], 0.0)
            ps_warm = ps1_pool.tile([128, 512], fp32, name="pswarm", tag="ps1")
            N_WARM = 28
            for i in range(N_WARM):
                nc.tensor.matmul(ps_warm[:], warm_w[:], warm_x[:],
                                 start=(i == 0), stop=(i == N_WARM - 1))
            # consolidated loads: straight contiguous copies, one DMA each
            # (the SP queue serializes descriptor generation, so count matters).
            nc.sync.dma_start(out=wqk_all[:], in_=wqkT)
            nc.sync.dma_start(out=x_all[:, 0:4096], in_=xT[:, 0:4096])
            nc.sync.dma_start(out=cs_t[:], in_=csF)
            nc.sync.dma_start(out=sm_t[:], in_=smalls)
            nc.sync.dma_start(out=wv_all[:], in_=wvT)
            nc.sync.dma_start(out=x_all[:, 4096:8192], in_=xT[:, 4096:8192])
            nc.sync.dma_start(out=x_all[:, 8192:16384], in_=xT[:, 8192:16384])
            nc.sync.dma_start(out=pw_all[:], in_=pwT)
            for kt in range(NKT):
                nc.vector.memset(
                    vaug_t[kt].rearrange("p (h w) -> p h w", w=HD + 1)[:, :, HD:HD + 1],
                    1.0)
            # causal triangle mask for diagonal 128x128 blocks: 1 where q >= p,
            # duplicated side by side so one strided mul masks both heads.
            nc.vector.memset(msk_t[:], 1.0)
            for mh in range(2):
                nc.gpsimd.affine_select(
                    out=msk_t[:, mh * 128:(mh + 1) * 128],
                    in_=msk_t[:, mh * 128:(mh + 1) * 128],
                    compare_op=mybir.AluOpType.is_ge, fill=0.0,
                    base=0, pattern=[[1, 128]], channel_multiplier=-1)

            # --- phase-1 -----------------------------------------------------
            def emit_qk(st, ot):
                """One (o 128, s 512) q^T/k^T tile: matmul + fused bias + RoPE.

                Partition layout (L'): [hA d0:32 | hB d0:32 | hA d32:64 |
                hB d32:64]; rotation partner = p ^ 64.  out = (ps+b)*cos +
                swapped((ps+b)*sin'); sin' carries the sign at the read index.
                """
                s0 = st * 512
                ps = ps1_pool.tile([128, 512], fp32, name=f"psA{st}_{ot}", tag="ps1")
                for dc in range(NDC):
                    nc.tensor.matmul(ps[:], wqk_t[dc][:, ot * 128:(ot + 1) * 128],
                                     x_slice(dc, st),
                                     start=(dc == 0), stop=(dc == NDC - 1))
                if ROPE_ACT:
                    # bias-add on ACT (slack engine in phase1 windows); DVE
                    # does 3 fp16 tensor_tensor passes + the band-swap copies.
                    t = p1sb.tile([128, 512], fp16, name=f"t{st}_{ot}", tag="t")
                    nc.scalar.add(t[:], ps[:], sm_t[:, ot:ot + 1])
                    m2s = p1sb.tile([128, 512], fp16, name=f"m2s{st}_{ot}",
                                    tag="m2s")
                    nc.vector.tensor_mul(m2s[:], t[:], cs_t[:, S + s0:S + s0 + 512])
                else:
                    m2s = p1sb.tile([128, 512], fp16, name=f"m2s{st}_{ot}",
                                    tag="m2s")
                    nc.vector.scalar_tensor_tensor(
                        m2s[:], ps[:], sm_t[:, ot:ot + 1],
                        cs_t[:, S + s0:S + s0 + 512], op0=ADD, op1=MULT)
                m2 = p1sb.tile([128, 512], fp16, name=f"m2{st}_{ot}", tag="m2")
                for band in range(4):
                    dstp = band * 32
                    srcp = (band ^ 1) * 32
                    nc.vector.tensor_copy(m2[dstp:dstp + 32, :], m2s[srcp:srcp + 32, :])
                qs = qk_t[ot][:, s0:s0 + 512]
                if ROPE_ACT:
                    nc.vector.tensor_mul(qs, t[:], cs_t[:, s0:s0 + 512])
                else:
                    nc.vector.scalar_tensor_tensor(
                        qs, ps[:], sm_t[:, ot:ot + 1], cs_t[:, s0:s0 + 512],
                        op0=ADD, op1=MULT)
                nc.vector.tensor_add(qs, qs, m2[:])

            def emit_v(st, ss):
                ps = ps1_pool.tile([128, O_V], fp32, name=f"psB{st}_{ss}", tag="ps1")
                for dc in range(NDC):
                    nc.tensor.matmul(ps[:],
                                     x_slice(dc, st)[:, ss * 128:(ss + 1) * 128],
                                     wv_t[dc][:], start=(dc == 0), stop=(dc == NDC - 1))
                vt = vaug_t[st * 4 + ss]
                nc.vector.tensor_copy(
                    vt.rearrange("p (h w) -> p h w", w=HD + 1)[:, :, 0:HD],
                    ps.rearrange("p (h w) -> p h w", w=HD))

            def phase1_chunks(st):
                ch = []
                for ot in range(4):
                    ch.append(lambda ot=ot: emit_qk(st, ot))
                for ss in range(4):
                    ch.append(lambda ss=ss: emit_v(st, ss))
                return ch

            # --- attention ---------------------------------------------------
            def attention(qt, filler, tail_proj=None):
                q0 = qt * 512
                kmax = (q0 + 512) // 128
                n_fill_slots = 2 * (kmax + 2)
                fill_every = max(1, n_fill_slots // max(1, len(filler))) if filler else 0
                step = 0

                def maybe_fill():
                    nonlocal step
                    step += 1
                    if filler and fill_every and step % fill_every == 0:
                        filler.pop(0)()

                for hp in range(2):
                    q_tile = qk_t[hp]
                    k_tile = qk_t[2 + hp]
                    hA, hB = 2 * hp, 2 * hp + 1
                    psO_A = psOA_pool.tile([HD + 1, 512], fp32, name=f"psOA{hp}_{qt}",
                                           tag="psOA")
                    psO_B = psOB_pool.tile([HD + 1, 512], fp32, name=f"psOB{hp}_{qt}",
                                           tag="psOB")
                    e_tiles = [None] * kmax

                    def emit_st(kt):
                        k0 = kt * 128
                        lo = max(0, k0 - q0)
                        psS = psS_pool.tile([128, 1024], fp32, name=f"psS{hp}_{qt}_{kt}",
                                            tag="psS")
                        nc.tensor.matmul(psS[:, lo:512], k_tile[0:64, k0:k0 + 128],
                                         q_tile[0:64, q0 + lo:q0 + 512],
                                         start=True, stop=True, tile_position=(0, 0))
                        nc.tensor.matmul(psS[:, 512 + lo:1024], k_tile[64:128, k0:k0 + 128],
                                         q_tile[64:128, q0 + lo:q0 + 512],
                                         start=True, stop=True, tile_position=(64, 0))
                        e = p2sb.tile([128, 1024], fp16, name=f"e{hp}_{qt}_{kt}", tag="e")
                        nc.scalar.activation(e[:, lo:1024], psS[:, lo:1024],
                                             Exp, scale=float(SCALE))
                        if k0 > q0 - 128:
                            ev = e.rearrange("p (h w) -> p h w", h=2)[:, :, lo:lo + 128]
                            mv = msk_t.rearrange("p (h w) -> p h w", h=2)
                            nc.vector.tensor_mul(ev, ev, mv)
                        e_tiles[kt] = (e, lo)

                    def emit_pv(kt):
                        e, lo = e_tiles[kt]
                        nc.tensor.matmul(psO_A[:, lo:512],
                                         vaug_t[kt][:, hA * 65:hA * 65 + 65],
                                         e[:, lo:512], start=(kt == 0),
                                         stop=(kt == kmax - 1), skip_group_check=True)
                        nc.tensor.matmul(psO_B[:, lo:512],
                                         vaug_t[kt][:, hB * 65:hB * 65 + 65],
                                         e[:, 512 + lo:1024], start=(kt == 0),
                                         stop=(kt == kmax - 1), skip_group_check=True)

                    def norm_chunk(j0, w):
                        c0, c1 = j0 * 128, (j0 + w) * 128
                        for idx, psO in ((0, psO_A), (1, psO_B)):
                            lr = nrm.tile([1, 512], fp32,
                                          name=f"lr{hp}_{qt}_{idx}_{j0}", tag="lr")
                            nc.vector.tensor_copy(lr[0:1, 0:c1 - c0],
                                                  psO[HD:HD + 1, c0:c1])
                            rec = nrm.tile([1, 512], fp32,
                                           name=f"rec{hp}_{qt}_{idx}_{j0}", tag="rec")
                            nc.vector.reciprocal_approx_fast(
                                rec[0:1, 0:c1 - c0], lr[0:1, 0:c1 - c0])
                            rbc = nrm.tile([64, 512], fp32,
                                           name=f"rbc{hp}_{qt}_{idx}_{j0}", tag="rbc")
                            nc.gpsimd.partition_broadcast(
                                rbc[:, 0:c1 - c0], rec[0:1, 0:c1 - c0])
                            nc.vector.tensor_mul(
                                outT_t[hp][64 * idx:64 * idx + 64, q0 + c0:q0 + c1],
                                psO[0:64, c0:c1], rbc[:, 0:c1 - c0])

                    # scores lookahead: PE gets PRE tiles + fills of
                    # psO-independent work before the first PV touches psO.
                    emit_st(0)
                    emit_st(1)
                    maybe_fill()
                    if kmax > 2:
                        emit_st(2)
                    maybe_fill()
                    if kmax > 3:
                        emit_st(3)
                    for kt in range(kmax):
                        if kt + PRE < kmax:
                            emit_st(kt + PRE)
                        emit_pv(kt)
                        maybe_fill()
                    if tail_proj is not None and hp == 1:
                        # last q-block: normalize in halves with this block's
                        # proj interleaved, so the PE projects half 1 while the
                        # DVE/gpsimd normalize half 2 (nothing else follows).
                        norm_chunk(0, 2)
                        tail_proj[0]()
                        tail_proj[1]()
                        norm_chunk(2, 2)
                        tail_proj[2]()
                        tail_proj[3]()
                    else:
                        norm_chunk(0, 4)
                while filler:
                    filler.pop(0)()

            # --- output projection -------------------------------------------
            def emit_proj(st128, on_act=False):
                po = p3sb.tile([128, D], fp16, name=f"po{st128}", tag="po")
                for jt in range(2):
                    ps = ps1_pool.tile([128, 512], fp32, name=f"psP{st128}_{jt}",
                                       tag="ps1")
                    for dc in range(2):
                        nc.tensor.matmul(ps[:], outT_t[dc][:, st128 * 128:(st128 + 1) * 128],
                                         pw_t[dc][:, jt * 512:(jt + 1) * 512],
                                         start=(dc == 0), stop=(dc == 1))
                    # mid-kernel evacuations go to DVE (ACT is exp-bound there);
                    # the tail chunks use ACT, which is idle once exp is done.
                    if on_act:
                        nc.scalar.copy(po[:, jt * 512:(jt + 1) * 512], ps[:])
                    else:
                        nc.vector.tensor_copy(po[:, jt * 512:(jt + 1) * 512], ps[:])
                nc.sync.dma_start(
                    out=outP[st128 * 128:(st128 + 1) * 128, :], in_=po[:])

            def proj_chunks(qt, on_act=False):
                return [lambda sst=sst: emit_proj(qt * 4 + sst, on_act)
                        for sst in range(4)]

            # --- main schedule ----------------------------------------------
            # Filler placement matched to per-window exp time (grows with qt):
            # early windows get only the mandatory phase1(qt+1); proj chunks
            # are deferred to the late, exp-heavy windows so the PE has work
            # while ACT drains the large score blocks.  attention(0) hp=0 only
            # needs qk ot 0/2 + v, so ot 1/3 ride as leading fillers.
            ch0 = phase1_chunks(0)
            pre0 = [ch0[0], ch0[2]] + ch0[4:8]
            fill_plan = {
                0: [ch0[1], ch0[3]] + phase1_chunks(1),
                1: phase1_chunks(2),
                2: phase1_chunks(3) + proj_chunks(0),
                3: proj_chunks(1) + proj_chunks(2),
            }
            for ch in pre0:
                ch()
            for qt in range(NST):
                attention(qt, fill_plan[qt],
                          tail_proj=(proj_chunks(qt, on_act=True)
                                     if qt == NST - 1 else None))

    nc.compile()
    return nc


def _get_nc():
    global _NC_CACHE
    if _NC_CACHE is None:
        _NC_CACHE = _build_nc()
    return _NC_CACHE


def _prep_in_maps(x, qkv_w, qkv_b, proj_w, proj_b):
    x = np.asarray(x, dtype=np.float32)
    qkv_w = np.asarray(qkv_w, dtype=np.float32)
    qkv_b = np.asarray(qkv_b, dtype=np.float32)
    proj_w = np.asarray(proj_w, dtype=np.float32)

    # RoPE tables in the interleaved (L') layout: freq index = p % 32,
    # sign = +1 for p < 64 (d < 32), -1 for p >= 64 (d >= 32).
    freqs = (1.0 / (10000.0 ** (np.arange(HALF, dtype=np.float32) / HALF))).astype(np.float32)
    pos = np.arange(S, dtype=np.float32)
    ang = pos[:, None] * freqs[None, :]            # (S, 32)
    cos_m = np.cos(ang).astype(np.float32)
    sin_m = np.sin(ang).astype(np.float32)
    p = np.arange(128)
    cosF = np.ascontiguousarray(cos_m[:, p % HALF].T)            # (128, S)
    sign = np.where((p % HD) < HALF, 1.0, -1.0).astype(np.float32)
    sinF = np.ascontiguousarray((sin_m[:, p % HALF] * sign[None, :]).T)
    csF = np.concatenate([cosF, sinF], axis=1).astype(np.float16)

    global _BVADD
    _BVADD = proj_w @ qkv_b[2 * D:3 * D]

    def chunk_major(a, widths):
        # [D_rows, C] -> [128, sum(n_chunks*width)]: per seq/col chunk,
        # contraction-chunk-major, partition-contiguous (fast DMA layout)
        blocks = []
        off = 0
        for w in widths:
            nchunks = a.shape[0] // 128
            blk = a[:, off:off + w].reshape(nchunks, 128, w)
            blocks.append(blk.transpose(1, 0, 2).reshape(128, nchunks * w))
            off += w
        return np.ascontiguousarray(np.concatenate(blocks, axis=1)).astype(np.float16)

    xb_l = [chunk_major(x[b].T, [512, 512, 1024]) for b in range(B)]
    in_maps = []
    for c in range(NCORES):
        b, g = divmod(c, GROUPS)
        hs = np.arange(HPG) + HPG * g
        q_rows = np.concatenate([np.arange(HD * h, HD * h + HD) for h in hs])
        v_rows = q_rows + 2 * D
        qk_rows = np.concatenate([q_rows, q_rows + D])
        smalls = np.zeros((128, 8), dtype=np.float32)
        smalls[:, 0:4] = qkv_b[qk_rows].reshape(4, 128).T
        in_maps.append({
            "xT": xb_l[b],
            "wqkT": chunk_major(qkv_w[qk_rows, :].T, [O_QK]),
            "wvT": chunk_major(qkv_w[v_rows, :].T, [O_V]),
            "pwT": chunk_major(proj_w[:, q_rows].T, [D]),
            "csF": csF,
            "smalls": smalls,
        })
    return in_maps


def _gather(results, proj_b):
    # v-bias folded here: softmax weights sum to 1, so attention(v + bv) =
    # attention(v) + bv, and proj(attn + bv) = proj(attn) + proj_w @ bv.
    proj_b = np.asarray(proj_b, dtype=np.float32)
    out = np.empty((B, S, D), dtype=np.float32)
    add = proj_b + _BVADD
    for b in range(B):
        acc = np.zeros((S, D), dtype=np.float32)
        for g in range(GROUPS):
            acc += results[b * GROUPS + g]["out_partial"].astype(np.float32)
        out[b] = acc + add[None, :]
    return out


def _run(x, qkv_w, qkv_b, proj_w, proj_b, trace=False):
    from concourse import bass_utils
    nc = _get_nc()
    in_maps = _prep_in_maps(x, qkv_w, qkv_b, proj_w, proj_b)
    res = bass_utils.run_bass_kernel_spmd(nc, in_maps, core_ids=list(range(NCORES)),
                                          trace=trace)
    return _gather(res.results, proj_b), res


def kernel(x, qkv_w, qkv_b, proj_w, proj_b):
    out, _ = _run(x, qkv_w, qkv_b, proj_w, proj_b, trace=False)
    return out

